# revision 1
# baseline (speedup 1.0000x reference)
"""Trainium2 Bass kernel: AdditiveAttention-style scoring head.

Computes, for x:(B,N,D), W1/W2:(A,D), b1/b2:(A,), Wout:(A,), bout:(1,):
    x1 = x @ W1.T + b1                       (B,N,A)
    x2 = x @ W2.T + b2                       (B,N,A)
    out[b,i-1,j] = sum_a Wout[a]*tanh(x1[b,j,a] + x2[b,i,a]) + bout,  i=1..N-1

Sharding: data-parallel over batch B across 8 NeuronCores (B/8=4 per core),
weights replicated, no collectives. Per core the 33M-element tanh stream is
the roofline (ACT engine, 128 lanes @1.2GHz); the broadcast add runs on DVE
(bf16 2x mode via duplicated-pair APs), and the A-reduction against Wout runs
on the TensorEngine as accumulating K=128 matmuls whose zero-padded
stationary operand routes each 512-col slice to its own PSUM partition
(even/odd slices on two banks so accumulation chains don't serialize); a
fused DVE tensor_scalar stages PSUM->SBUF adding bout on the way out.
"""
import sys
import numpy as np

if "/opt/trn_rl_repo" not in sys.path:
    sys.path.insert(0, "/opt/trn_rl_repo")

B, N, D, A = 32, 128, 512, 512
NCORES = 8
BPC = B // NCORES      # batches per core
KC = D // 128          # contraction chunks for the input matmuls
MC = A // 128          # a-chunks (partition dim of the fused stage)
IB = 64                # i-rows per pipeline block
NIB = N // IB          # i-blocks per batch (covers i=0..N-1; i=0 dropped at DMA)
F = IB * N             # free elements per (b, iblock) tile
MMN = 512              # matmul free dim (one psum bank)
G = F // MMN           # output slices per iblock (8): even/odd across 2 psum banks
GH = G // 2            # slices per bank (4) = psum rows used

_CACHE = {}


def _build_nc():
    import concourse.bass as bass
    import concourse.bacc as bacc
    import concourse.mybir as mybir
    from concourse import tile

    f32 = mybir.dt.float32
    bf16 = mybir.dt.bfloat16
    AF = mybir.ActivationFunctionType

    nc = bacc.Bacc(None, target_bir_lowering=False)

    xT = nc.declare_dram_parameter("xT", [D, BPC * N], bf16, isOutput=False)
    # w?tc[mc, d, j] = W?[mc*128+j, d] — a-chunk-major so chunk 0's weights
    # land first and the main pipeline starts early
    w1t = nc.declare_dram_parameter("w1t", [MC, D, 128], bf16, isOutput=False)
    w2t = nc.declare_dram_parameter("w2t", [MC, D, 128], bf16, isOutput=False)
    b1c = nc.declare_dram_parameter("b1c", [128, MC], f32, isOutput=False)
    b2c = nc.declare_dram_parameter("b2c", [128, MC], f32, isOutput=False)
    # woutpad: per-(c, r) stationary [128, GH] tiles, nonzero only in column r
    # = Wout chunk c. An MM with this lhsT routes its slice's reduction to
    # psum partition r (rows != r accumulate +0). The (c, r) tile is shared
    # by the even and odd slice 2r/2r+1 MMs (two different psum banks).
    woutpad = nc.declare_dram_parameter("woutpad", [128, MC * GH * GH], bf16, isOutput=False)
    boutp = nc.declare_dram_parameter("bout", [128, 1], f32, isOutput=False)
    out = nc.declare_dram_parameter("out", [BPC, (N - 1) * N], f32, isOutput=True)

    with tile.TileContext(nc) as tc:
        with (
            tc.tile_pool(name="const", bufs=1) as cpool,
            tc.tile_pool(name="xw", bufs=1) as xwpool,
            tc.tile_pool(name="x12", bufs=1) as xpool,
            tc.tile_pool(name="s", bufs=3) as spool,
            tc.tile_pool(name="t", bufs=5) as tpool,
            tc.tile_pool(name="stage", bufs=4) as stpool,
        ):
            # ---- PE warmup: dummy matmuls on junk data during the input DMA
            # window so the HAM clock-gate is at 8/8 when the real matmuls
            # arrive (net positive: cold setup matmuls cost more than the
            # warmup's PE-queue occupancy) ----
            warm = cpool.tile([128, MMN], bf16, tag="warm")
            nc.gpsimd.memset(warm[:, :], 0.25)
            with tc.tile_pool(name="psW", bufs=1, space=bass.MemorySpace.PSUM) as psW:
                wps = psW.tile([128, MMN], f32, tag="psW")
                for _ in range(9):
                    nc.tensor.matmul(wps[:, :], warm[:, 0:128], warm[:, :],
                                     start=True, stop=True)

            # ---- input loads (bf16); xT split across two DMA queues, weights
            # in mc-major order on the gpsimd queue so mc=0 lands first ----
            xT_sb = []
            for k in range(KC):
                tx = xwpool.tile([128, BPC * N], bf16, tag=f"xT{k}")
                eng = nc.sync if k % 2 == 0 else nc.scalar
                eng.dma_start(tx[:, :], xT[k * 128:(k + 1) * 128, :])
                xT_sb.append(tx)
            # One 3D-AP DMA per (matrix, a-chunk): SBUF [d', k*128+j] <-
            # DRAM w?t[m, k*128+d', j]; w2 on the gpsimd queue, w1 on scalar.
            w1_sb, w2_sb = [], []
            for m in range(MC):
                t2 = xwpool.tile([128, KC * 128], bf16, tag=f"w2{m}", name=f"w2_{m}")
                d2 = t2[:, :]
                dst2 = bass.AP(d2.tensor, d2.offset,
                               [[d2.ap[0][0], 128], [128, KC], [1, 128]])
                src2 = bass.AP(w2t[0, :, :].tensor, m * D * 128,
                               [[128, 128], [128 * 128, KC], [1, 128]])
                nc.gpsimd.dma_start(dst2, src2)
                w2_sb.append(t2)
                t1 = xwpool.tile([128, KC * 128], bf16, tag=f"w1{m}", name=f"w1_{m}")
                d1 = t1[:, :]
                dst1 = bass.AP(d1.tensor, d1.offset,
                               [[d1.ap[0][0], 128], [128, KC], [1, 128]])
                src1 = bass.AP(w1t[0, :, :].tensor, m * D * 128,
                               [[128, 128], [128 * 128, KC], [1, 128]])
                nc.scalar.dma_start(dst1, src1)
                w1_sb.append(t1)
            b1_sb = cpool.tile([128, MC], f32, tag="b1")
            nc.sync.dma_start(b1_sb[:, :], b1c[:, :])
            b2_sb = cpool.tile([128, MC], f32, tag="b2")
            nc.sync.dma_start(b2_sb[:, :], b2c[:, :])
            wout_sb = cpool.tile([128, MC * GH * GH], bf16, tag="wout")
            nc.sync.dma_start(wout_sb[:, :], woutpad[:, :])
            boutf = cpool.tile([128, 1], f32, tag="boutf")
            nc.sync.dma_start(boutf[:, :], boutp[:, :])

            x1_sb = [xpool.tile([128, BPC * N], bf16, tag=f"x1_{c}", name=f"x1_{c}") for c in range(MC)]
            x2d_sb = [xpool.tile([128, BPC * N * 2], bf16, tag=f"x2d_{c}", name=f"x2d_{c}") for c in range(MC)]

            # ---- x1/x2 = W @ x^T + b, in [a_chunk, (b,n)] layout, cast bf16.
            # Emitted lazily per chunk, interleaved with the first block's
            # TT/ACT ops so the DVE doesn't front-load all of setup before
            # the first tanh tile is produced.
            def emit_setup(m, narrow=False):
                # x2 chunk: bias-add + pair-duplication fused in one
                # PSUM-sourced op writing x2d[:, 2q+t] = x2[:, q] + b2.
                # narrow=True emits only batch-0's columns now (so the first
                # tanh can start early) and returns a closure for the rest.
                ps2 = psA.tile([128, BPC * N], f32, tag="psA", name=f"ps2_{m}")
                for k in range(KC):
                    nc.tensor.matmul(ps2[:, :], w2_sb[m][:, k * 128:(k + 1) * 128],
                                     xT_sb[k][:, :],
                                     start=(k == 0), stop=(k == KC - 1))
                psap = ps2[:, :]
                dst = x2d_sb[m][:, :]
                nw = N if narrow else BPC * N

                def dup(lo, n_):
                    in_ap = bass.AP(psap.tensor, psap.offset + lo,
                                    [[psap.ap[0][0], 128], [1, n_], [0, 2]])
                    out_ap = bass.AP(dst.tensor, dst.offset + 2 * lo,
                                     [[dst.ap[0][0], 128], [2, n_], [1, 2]])
                    nc.vector.tensor_scalar_add(out_ap, in_ap, b2_sb[:, m:m + 1])

                dup(0, nw)
                # x1 chunk
                ps1 = psA.tile([128, BPC * N], f32, tag="psA", name=f"ps1_{m}")
                for k in range(KC):
                    nc.tensor.matmul(ps1[:, :], w1_sb[m][:, k * 128:(k + 1) * 128],
                                     xT_sb[k][:, :],
                                     start=(k == 0), stop=(k == KC - 1))
                nc.vector.tensor_scalar_add(x1_sb[m][:, 0:nw], ps1[:, 0:nw],
                                            b1_sb[:, m:m + 1])
                if not narrow:
                    return None

                def rest():
                    dup(nw, BPC * N - nw)
                    nc.vector.tensor_scalar_add(x1_sb[m][:, nw:], ps1[:, nw:],
                                                b1_sb[:, m:m + 1])
                return rest

            # ---- main pipeline: DVE add -> ACT tanh -> PE reduce -> DMA out ----
            with (
                tc.tile_pool(name="psA", bufs=4, space=bass.MemorySpace.PSUM) as psA,
                tc.tile_pool(name="psO", bufs=4, space=bass.MemorySpace.PSUM) as psO,
            ):
                first = True
                rest_q = []
                for b in range(BPC):
                    # last batch ends with two half blocks so the post-ACT
                    # matmul/stage/DMA tail is shorter
                    blocks = [(k * IB, IB) for k in range(NIB)]
                    if b == BPC - 1:
                        i0L, nbL = blocks.pop()
                        blocks += [(i0L, nbL // 2), (i0L + nbL // 2, nbL // 2)]
                    for i0, nb in blocks:
                        fb = nb * N          # free elems this block
                        gh = fb // MMN // 2  # even/odd slice pairs
                        psE = psO.tile([GH, MMN], f32, tag="psO", name=f"psE_{b}_{i0}")
                        psF = psO.tile([GH, MMN], f32, tag="psO", name=f"psF_{b}_{i0}")
                        for c in range(MC):
                            # All chunks' setup is narrowed to batch-0 columns;
                            # each chunk's remainder runs two c-slots later so
                            # rests free psA slots just before the next chunk
                            # allocates, and the head-window DVE chain stays
                            # short (the rest columns aren't needed until b=1).
                            if first:
                                if c >= 2 and rest_q:
                                    rest_q.pop(0)()
                                rest_q.append(emit_setup(c, narrow=True))
                            elif b == 0 and i0 == IB and rest_q:
                                rest_q.pop(0)()
                            s = spool.tile([128, F], bf16, tag="s")
                            sap = s[:, :]
                            x1ap = x1_sb[c][:, b * N:(b + 1) * N]
                            in0 = bass.AP(x1ap.tensor, x1ap.offset,
                                          [[x1ap.ap[0][0], 128], [0, nb], [2, N // 2], [1, 2]])
                            x2ap = x2d_sb[c][:, :]
                            in1 = bass.AP(x2ap.tensor, x2ap.offset + (b * N + i0) * 2,
                                          [[x2ap.ap[0][0], 128], [2, nb], [0, N // 2], [1, 2]])
                            sout = bass.AP(sap.tensor, sap.offset,
                                           [[sap.ap[0][0], 128], [N, nb], [2, N // 2], [1, 2]])
                            tt = tpool.tile([128, F], bf16, tag="t")
                            if b == 0 and i0 == 0 and c <= 1:
                                # split the earliest tiles so the ACT engine
                                # starts sooner and stays fed
                                if c == 0:
                                    cuts = ((0, 16 * N), (16 * N, 40 * N), (40 * N, fb))
                                else:
                                    cuts = ((0, fb // 2), (fb // 2, fb))
                                for lo, hi in cuts:
                                    nbh = (hi - lo) // N
                                    in0h = bass.AP(in0.tensor, in0.offset,
                                                   [in0.ap[0], [0, nbh]] + in0.ap[2:])
                                    in1h = bass.AP(in1.tensor,
                                                   in1.offset + lo // N * 2,
                                                   [in1.ap[0], [2, nbh]] + in1.ap[2:])
                                    south = bass.AP(sout.tensor, sout.offset + lo,
                                                    [sout.ap[0], [N, nbh]] + sout.ap[2:])
                                    nc.vector.tensor_tensor(south, in0h, in1h,
                                                            mybir.AluOpType.add)
                                    nc.scalar.activation(tt[:, lo:hi], s[:, lo:hi],
                                                         AF.Tanh)
                            else:
                                nc.vector.tensor_tensor(sout, in0, in1,
                                                        mybir.AluOpType.add)
                                nc.scalar.activation(tt[:, :fb], s[:, :fb], AF.Tanh)
                            # slices 2r/2r+1 share lhsT (c, r) and go to the
                            # even/odd psum banks at partition r. On the very
                            # last (block, c) emit all E MMs first so stgE's
                            # staging/DMA overlaps the F MMs.
                            last_tail = (b == BPC - 1 and i0 == blocks[-1][0]
                                         and c == MC - 1)
                            order = ([(r, 0) for r in range(gh)]
                                     + [(r, 1) for r in range(gh)]) if last_tail                                 else [(r, p) for r in range(gh) for p in (0, 1)]
                            for r, p in order:
                                w0 = (c * GH + r) * GH
                                tgt = psE if p == 0 else psF
                                nc.tensor.matmul(tgt[:, :], wout_sb[:, w0:w0 + GH],
                                                 tt[:, (2 * r + p) * MMN:(2 * r + p + 1) * MMN],
                                                 start=(c == 0 and r == 0),
                                                 stop=(c == MC - 1 and r == gh - 1))
                        first = False
                        # stage PSUM->SBUF with +bout fused, then DMA out.
                        # stage row r of stgE/stgF = slice 2r / 2r+1.
                        stgE = stpool.tile([GH, MMN], f32, tag="stgE")
                        nc.vector.tensor_scalar_add(stgE[0:gh, :], psE[0:gh, :], boutf[0:gh, 0:1])
                        stgF = stpool.tile([GH, MMN], f32, tag="stgF")
                        nc.vector.tensor_scalar_add(stgF[0:gh, :], psF[0:gh, :], boutf[0:gh, 0:1])
                        o0 = i0 * N - N  # out-flat col of slice 0 (i=0 row dropped)
                        if i0 == 0:
                            # slice 0 = i rows 0..3; its first N cols are row i=0
                            nc.sync.dma_start(out[b:b + 1, 0:MMN - N], stgE[0:1, N:MMN])
                            ev = bass.AP(out[:, :].tensor,
                                         out[:, :].offset + b * (N - 1) * N + 2 * MMN - N,
                                         [[2 * MMN, gh - 1], [1, MMN]])
                            nc.sync.dma_start(ev, stgE[1:gh, :])
                        else:
                            ev = bass.AP(out[:, :].tensor,
                                         out[:, :].offset + b * (N - 1) * N + o0,
                                         [[2 * MMN, gh], [1, MMN]])
                            nc.sync.dma_start(ev, stgE[0:gh, :])
                        od = bass.AP(out[:, :].tensor,
                                     out[:, :].offset + b * (N - 1) * N + o0 + MMN,
                                     [[2 * MMN, gh], [1, MMN]])
                        nc.sync.dma_start(od, stgF[0:gh, :])

    nc.finalize()
    return nc


def _get_nc():
    if "nc" not in _CACHE:
        _CACHE["nc"] = _build_nc()
    return _CACHE["nc"]


def _prep_in_maps(x, W1, b1, W2, b2, Wout, bout):
    import ml_dtypes
    f = np.float32
    bf = ml_dtypes.bfloat16
    w1t = np.ascontiguousarray(
        np.asarray(W1, f).reshape(MC, 128, D).transpose(0, 2, 1).astype(bf))
    w2t = np.ascontiguousarray(
        np.asarray(W2, f).reshape(MC, 128, D).transpose(0, 2, 1).astype(bf))
    b1v = np.ascontiguousarray(np.asarray(b1, f).reshape(MC, 128).T)
    b2v = np.ascontiguousarray(np.asarray(b2, f).reshape(MC, 128).T)
    Wo = np.asarray(Wout, f)
    wop = np.zeros((128, MC * GH * GH), f)  # built f32, sent bf16
    for c in range(MC):
        for r in range(GH):
            wop[:, (c * GH + r) * GH + r] = Wo[c * 128:(c + 1) * 128]
    bov = np.full((128, 1), np.asarray(bout, f).reshape(()), f)
    x = np.asarray(x, f)
    in_maps = []
    for ci in range(NCORES):
        xs = x[ci * BPC:(ci + 1) * BPC]
        xTi = np.ascontiguousarray(
            xs.transpose(2, 0, 1).reshape(D, BPC * N).astype(bf))
        in_maps.append({
            "xT": xTi, "w1t": w1t, "w2t": w2t,
            "b1c": b1v, "b2c": b2v, "woutpad": wop.astype(bf), "bout": bov,
        })
    return in_maps


def _run(x, W1, b1, W2, b2, Wout, bout, trace=False):
    from concourse.bass_utils import run_bass_kernel_spmd

    nc = _get_nc()
    in_maps = _prep_in_maps(x, W1, b1, W2, b2, Wout, bout)
    res = run_bass_kernel_spmd(nc, in_maps, core_ids=list(range(NCORES)), trace=trace)
    outs = [np.asarray(res.results[ci]["out"]).reshape(BPC, N - 1, N)
            for ci in range(NCORES)]
    full = np.concatenate(outs, axis=0).astype(np.float32)
    return full, res


def kernel(x, W1, b1, W2, b2, Wout, bout):
    full, _ = _run(x, W1, b1, W2, b2, Wout, bout, trace=False)
    return full

